# revision 38
# baseline (speedup 1.0000x reference)
"""AttnBlock (GroupNorm -> QKV 1x1 -> full attention over 1024 tokens -> out-proj
+ residual) for x [32, 512, 32, 32] f32, distributed data-parallel over 8
NeuronCores (4 samples per core, weights replicated).

Per-core single-NC Bass/Tile kernel, bf16 TensorE compute, f32 softmax stats:
  - GroupNorm via per-channel bn_stats/bn_aggr + tiny selector matmuls for the
    cross-partition group reduce. Sample b+1's norm chain is issued mid-way
    through sample b's attention so its two tiny PE matmuls embed in the
    matmul stream without stalling it.
  - Q,K: [d, hw] layout; V computed transposed ([hw, d]) directly by swapping
    matmul operands.
  - Scores computed TRANSPOSED (ST[j,i] = sum_d K[d,j] Q[d,i]) so that
    P~ = exp(ST*c) has the contraction axis j on partitions: PV needs no
    transposes. Softmax denominator: DVE pairwise-add tree over the 8 exp
    tiles (f32), then one all-ones fp32 stationary matmul which both reduces
    the remaining 128 partitions and replicates the row-sum across all
    partitions; division is folded into the PV psum->sbuf copy via
    reciprocal_approx_fast.
  - Matmul loops are ordered so consecutive matmuls share the stationary
    operand (one LDWEIGHTS per two matmuls where possible).
"""

import os
import sys

import numpy as np

sys.path.insert(0, "/opt/trn_rl_repo")

import ml_dtypes  # noqa: E402

import concourse.bass as bass  # noqa: E402
import concourse.tile as tile  # noqa: E402
from concourse import bacc, mybir  # noqa: E402

P = 128
B_FULL, C, H, W = 32, 512, 32, 32
HW = H * W            # 1024 tokens
N_CORES = 8
NB = B_FULL // N_CORES  # 4 samples per core
NT = C // P           # 4 channel tiles
NJ = HW // P          # 8 token tiles
NGROUPS = 32
GS = C // NGROUPS     # 16 channels per group
G_PER_TILE = P // GS  # 8 groups per 128-channel tile
EPS = 1e-6
CINV = float(C) ** -0.5

f32 = mybir.dt.float32
bf16 = mybir.dt.bfloat16
ALU = mybir.AluOpType
ACT = mybir.ActivationFunctionType


def build_nc(zero_qk_bias=False):
    """Build the single-core Bass graph (SPMD: same graph on all 8 cores).

    zero_qk_bias: when bq/bk are all-zero (true for this problem's
    setup_inputs), the Q/K psum->sbuf copies use the table-free Copy
    activation instead of Identity+bias, avoiding ACT table thrash with Exp.
    """
    nc = bacc.Bacc("TRN2", target_bir_lowering=False, debug=False)

    x_d = nc.dram_tensor("x", [NB, C, HW], f32, kind="ExternalInput")
    wq_d = nc.dram_tensor("wq", [P, NT, C], bf16, kind="ExternalInput")
    wk_d = nc.dram_tensor("wk", [P, NT, C], bf16, kind="ExternalInput")
    wv_d = nc.dram_tensor("wv", [P, NT, C], bf16, kind="ExternalInput")
    wt_d = nc.dram_tensor("wt", [P, NT, C], bf16, kind="ExternalInput")
    # packed per-partition bias columns: [:, 0, :]=bq, [:, 1, :]=bk, [:, 2, :]=bt
    bqkt_d = nc.dram_tensor("bqkt", [P, 3, NT], f32, kind="ExternalInput")
    bv_d = nc.dram_tensor("bv_rep", [P, C], f32, kind="ExternalInput")
    # gn affine columns: [:, 0, :]=gamma, [:, 1, :]=beta
    gab_d = nc.dram_tensor("gn_ab", [P, 2, NT], f32, kind="ExternalInput")
    # block-diagonal group-average matrix: GG[k,p] = 1/16 iff k//16 == p//16
    gg_d = nc.dram_tensor("gg", [P, P], f32, kind="ExternalInput")
    out_d = nc.dram_tensor("out", [NB, C, HW], f32, kind="ExternalOutput")

    with tile.TileContext(nc) as tc:
        with (
            tc.tile_pool(name="consts", bufs=1) as consts,
            tc.tile_pool(name="hp", bufs=2) as hp,
            tc.tile_pool(name="qkv", bufs=1) as qkvp,
            tc.tile_pool(name="est", bufs=1) as estp,
            tc.tile_pool(name="op", bufs=1) as op,
            tc.tile_pool(name="outp", bufs=3) as outp,
            tc.tile_pool(name="small", bufs=2) as small,
            tc.tile_pool(name="sump", bufs=1) as sump,
            tc.tile_pool(name="psb", bufs=2, space="PSUM") as psb,
            tc.tile_pool(name="psq", bufs=4, space="PSUM") as psq,
        ):
            # ---- x[0] gates everything: one tile per DMA queue (4 parallel
            #      rings), then weights/consts sequenced by first-use time.
            x_sb = consts.tile([P, NB, NT, HW], f32, tag="x")
            wq_sb = consts.tile([P, NT, C], bf16, tag="wq")
            wk_sb = consts.tile([P, NT, C], bf16, tag="wk")
            wv_sb = consts.tile([P, NT, C], bf16, tag="wv")
            wt_sb = consts.tile([P, NT, C], bf16, tag="wt")
            gab_sb = consts.tile([P, 2, NT], f32, tag="gab")
            gg_sb = consts.tile([P, P], f32, tag="gg")
            bqkt_sb = consts.tile([P, 3, NT], f32, tag="bqkt")
            bv_sb = consts.tile([P, C], f32, tag="bv")

            def x0_quarters(eng, t):
                for h0 in (0, 256, 512, 768):
                    eng.dma_start(out=x_sb[:, 0, t, h0:h0 + 256],
                                  in_=x_d[0, t * P:(t + 1) * P, h0:h0 + 256])

            # sync queue (fastest start): tiny GN consts, x0 tiles 0,3,
            # then x[1], x[3]
            nc.sync.dma_start(out=gg_sb[:, :], in_=gg_d[:, :])
            nc.sync.dma_start(out=gab_sb[:, :, :], in_=gab_d[:, :, :])
            x0_quarters(nc.sync, 0)
            x0_quarters(nc.sync, 3)
            for b in (1, 3):
                for t in range(NT):
                    nc.sync.dma_start(out=x_sb[:, b, t, :],
                                      in_=x_d[b, t * P:(t + 1) * P, :])
            # scalar queue: x0 tile 2, then wq (needed ~15us), wk (~18us)
            x0_quarters(nc.scalar, 2)
            nc.scalar.dma_start(out=wq_sb[:, :, :], in_=wq_d[:, :, :])
            nc.scalar.dma_start(out=wk_sb[:, :, :], in_=wk_d[:, :, :])
            # gpsimd queue: x0 tile 1, x[2], then late weights
            x0_quarters(nc.gpsimd, 1)
            nc.gpsimd.dma_start(out=bqkt_sb[:, :, :], in_=bqkt_d[:, :, :])
            for t in range(NT):
                nc.gpsimd.dma_start(out=x_sb[:, 2, t, :],
                                    in_=x_d[2, t * P:(t + 1) * P, :])
            nc.gpsimd.dma_start(out=wv_sb[:, :, :], in_=wv_d[:, :, :])
            nc.gpsimd.dma_start(out=bv_sb[:, :], in_=bv_d[:, :])
            nc.gpsimd.dma_start(out=wt_sb[:, :, :], in_=wt_d[:, :, :])
            ones_sb = consts.tile([P, P], f32, tag="ones")
            nc.vector.memset(ones_sb[:, :], 1.0)
            magic_sb = consts.tile([P, NT], mybir.dt.int32, tag="magic")
            nc.vector.memset(magic_sb[:, :], 0x5F3759DF)

            a_all = consts.tile([P, NB, NT], f32, tag="a_all")
            b_all = consts.tile([P, NB, NT], f32, tag="b_all")

            # PE warm-up: harmless matmuls on the memset ones tile (no DMA
            # dependency) so the HAM clock-gate is released before the real
            # stream and stays released until the first QK matmuls.
            warm_ps = psq.tile([P, 512], f32, tag="qkv")
            for w in range(16):
                nc.tensor.matmul(
                    warm_ps[:, 0:P], ones_sb[:, :], ones_sb[:, :],
                    start=(w == 0), stop=(w == 15),
                )

            def gn_stats(b, after=None):
                """bn stats -> per-channel (mean, Ex2) packed in mv."""
                mv = small.tile([P, NT, 2], f32, tag="mv")
                for t in range(NT):
                    st6 = small.tile([P, 2, 6], f32, tag="st6")
                    i0 = nc.vector.bn_stats(out=st6[:, 0, :], in_=x_sb[:, b, t, 0:512])
                    i1 = nc.vector.bn_stats(out=st6[:, 1, :], in_=x_sb[:, b, t, 512:1024])
                    nc.vector.bn_aggr(out=mv[:, t, :], in_=st6[:, :, :])
                    if after is not None:
                        # keep these off the DVE stream until the previous
                        # sample's applies have been scheduled
                        tile.add_dep_helper(i0.ins, after.ins, sync=False,
                                            reason="gn stats after prev apply")
                        tile.add_dep_helper(i1.ins, after.ins, sync=False,
                                            reason="gn stats after prev apply")
                msq = small.tile([P, NT], f32, tag="msq")
                nc.vector.tensor_mul(msq[:, :], mv[:, :, 0], mv[:, :, 0])
                nc.vector.tensor_add(mv[:, :, 1], mv[:, :, 1], msq[:, :])
                return mv

            def gn_affine(b, mv, use_act_sqrt=False):
                """fused group-avg+broadcast matmul, then form per-channel A/B."""
                bc_ps = psq.tile([P, 512], f32, tag="qkv")
                nc.tensor.matmul(bc_ps[:, :NT * 2], gg_sb[:, :], mv[:, :, :],
                                 start=True, stop=True)
                bc = small.tile([P, NT, 2], f32, tag="bcs")
                nc.vector.tensor_copy(bc[:, :, :], bc_ps[:, 0:NT * 2])
                vb = small.tile([P, NT], f32, tag="vb")
                nc.vector.tensor_mul(vb[:, :], bc[:, :, 0], bc[:, :, 0])
                nc.vector.tensor_sub(vb[:, :], bc[:, :, 1], vb[:, :])
                if use_act_sqrt:
                    # sample 0 (pre-exp): the shorter ACT chain wins and its
                    # Sqrt table load cannot evict a not-yet-loaded Exp table
                    nc.vector.tensor_scalar_add(vb[:, :], vb[:, :], EPS)
                    nc.scalar.sqrt(vb[:, :], vb[:, :])
                    nc.vector.reciprocal(vb[:, :], vb[:, :])
                    y1 = vb
                else:
                    nc.vector.tensor_scalar_add(vb[:, :], vb[:, :], EPS)
                    # rstd = rsqrt(var+eps): fast-inverse-sqrt + 2 Newton steps
                    # (all-DVE: keeps Sqrt off ACT so it never evicts Exp)
                    ii = small.tile([P, NT], mybir.dt.int32, tag="ii")
                    nc.vector.tensor_scalar(
                        out=ii[:, :], in0=vb.bitcast(mybir.dt.int32)[:, :],
                        scalar1=1, scalar2=None, op0=ALU.arith_shift_right)
                    nc.vector.tensor_tensor(ii[:, :], magic_sb[:, :], ii[:, :],
                                            op=ALU.subtract)
                    y0 = ii.bitcast(f32)
                    yt = small.tile([P, NT], f32, tag="yt")
                    y1 = small.tile([P, NT], f32, tag="y1")
                    nc.vector.tensor_mul(yt[:, :], vb[:, :], y0[:, :])
                    nc.vector.tensor_mul(yt[:, :], yt[:, :], y0[:, :])
                    nc.vector.tensor_scalar(out=yt[:, :], in0=yt[:, :], scalar1=-0.5,
                                            scalar2=1.5, op0=ALU.mult, op1=ALU.add)
                    nc.vector.tensor_mul(y1[:, :], y0[:, :], yt[:, :])
                    nc.vector.tensor_mul(yt[:, :], vb[:, :], y1[:, :])
                    nc.vector.tensor_mul(yt[:, :], yt[:, :], y1[:, :])
                    nc.vector.tensor_scalar(out=yt[:, :], in0=yt[:, :], scalar1=-0.5,
                                            scalar2=1.5, op0=ALU.mult, op1=ALU.add)
                    nc.vector.tensor_mul(y1[:, :], y1[:, :], yt[:, :])
                tmp = small.tile([P, NT], f32, tag="tmpab")
                nc.vector.tensor_mul(a_all[:, b, :], y1[:, :], gab_sb[:, 0, :])
                nc.vector.tensor_mul(tmp[:, :], bc[:, :, 0], a_all[:, b, :])
                nc.vector.tensor_sub(b_all[:, b, :], gab_sb[:, 1, :], tmp[:, :])

            def apply_h(b, after=None):
                """h = x*A + B (bf16)"""
                h = hp.tile([P, NT, HW], bf16, tag="h")
                last = None
                for t in range(NT):
                    last = nc.vector.tensor_scalar(
                        out=h[:, t, :], in0=x_sb[:, b, t, :],
                        scalar1=a_all[:, b, t:t + 1], scalar2=b_all[:, b, t:t + 1],
                        op0=ALU.mult, op1=ALU.add,
                    )
                    if after is not None:
                        # the applies must not jump ahead of this sample's
                        # O-scales on the in-order DVE stream
                        tile.add_dep_helper(last.ins, after.ins, sync=False,
                                            reason="apply after O-scales")
                return h, last

            # ---- sample 0: per-tile GN chains, each gated only on its own
            #      tile's DMA, so the first QK matmuls start ~8us earlier ----
            h_cur = hp.tile([P, NT, HW], bf16, tag="h")
            last_apply = None
            for t in range(NT):
                st6 = small.tile([P, 2, 6], f32, tag="st6")
                nc.vector.bn_stats(out=st6[:, 0, :], in_=x_sb[:, 0, t, 0:512])
                nc.vector.bn_stats(out=st6[:, 1, :], in_=x_sb[:, 0, t, 512:1024])
                mvt = small.tile([P, 2], f32, tag="mvt")
                nc.vector.bn_aggr(out=mvt[:, :], in_=st6[:, :, :])
                msq = small.tile([P, 1], f32, tag="msq0")
                nc.vector.tensor_mul(msq[:, :], mvt[:, 0:1], mvt[:, 0:1])
                nc.vector.tensor_add(mvt[:, 1:2], mvt[:, 1:2], msq[:, :])
                bc_ps = psq.tile([P, 512], f32, tag="qkv")
                nc.tensor.matmul(bc_ps[:, 0:2], gg_sb[:, :], mvt[:, :],
                                 start=True, stop=True)
                bct = small.tile([P, 2], f32, tag="bct")
                nc.vector.tensor_copy(bct[:, :], bc_ps[:, 0:2])
                vbt = small.tile([P, 1], f32, tag="vbt")
                nc.vector.tensor_mul(vbt[:, :], bct[:, 0:1], bct[:, 0:1])
                nc.vector.tensor_sub(vbt[:, :], bct[:, 1:2], vbt[:, :])
                nc.vector.tensor_scalar_add(vbt[:, :], vbt[:, :], EPS)
                nc.scalar.sqrt(vbt[:, :], vbt[:, :])
                nc.vector.reciprocal(vbt[:, :], vbt[:, :])
                tmp0 = small.tile([P, 1], f32, tag="tmp0")
                nc.vector.tensor_mul(a_all[:, 0, t:t + 1], vbt[:, :],
                                     gab_sb[:, 0, t:t + 1])
                nc.vector.tensor_mul(tmp0[:, :], bct[:, 0:1], a_all[:, 0, t:t + 1])
                nc.vector.tensor_sub(b_all[:, 0, t:t + 1], gab_sb[:, 1, t:t + 1],
                                     tmp0[:, :])
                last_apply = nc.vector.tensor_scalar(
                    out=h_cur[:, t, :], in0=x_sb[:, 0, t, :],
                    scalar1=a_all[:, 0, t:t + 1], scalar2=b_all[:, 0, t:t + 1],
                    op0=ALU.mult, op1=ALU.add,
                )

            for b in range(NB):
                h_sb = h_cur

                # ---- Q, K projections: [d, hw]; lhsT reused across n-halves ----
                q_sb = qkvp.tile([P, NT, HW], bf16, tag="q")
                k_sb = qkvp.tile([P, NT, HW], bf16, tag="k")
                # Q psum->sbuf copies on ACT (overlap the K matmuls); K copies
                # on the then-idle DVE so ACT's exp stream starts immediately
                # once the score matmuls begin.
                for dst_sb, w_sb, bias_idx in ((q_sb, wq_sb, 0), (k_sb, wk_sb, 1)):
                    for dm in range(NT):
                        ps0 = psq.tile([P, 512], f32, tag="qkv")
                        ps1 = psq.tile([P, 512], f32, tag="qkv")
                        for kc in range(NT):
                            lhsT = w_sb[:, kc, dm * P:(dm + 1) * P]
                            st, sp = (kc == 0), (kc == NT - 1)
                            nc.tensor.matmul(ps0[:, :], lhsT,
                                             h_sb[:, kc, 0:512], start=st, stop=sp)
                            nc.tensor.matmul(ps1[:, :], lhsT,
                                             h_sb[:, kc, 512:1024], start=st, stop=sp)
                        for ps, nsl in ((ps0, slice(0, 512)), (ps1, slice(512, 1024))):
                            if zero_qk_bias:
                                nc.scalar.copy(dst_sb[:, dm, nsl], ps[:, :])
                            else:
                                nc.scalar.add(dst_sb[:, dm, nsl], ps[:, :],
                                              bqkt_sb[:, bias_idx, dm:dm + 1])
                # ---- V transposed: VT[j, d] ----
                vt_sb = qkvp.tile([P, NJ, C], bf16, tag="vt")
                for jm in range(NJ):
                    ps = psq.tile([P, 512], f32, tag="qkv")
                    for kc in range(NT):
                        nc.tensor.matmul(
                            ps[:, :],
                            h_sb[:, kc, jm * P:(jm + 1) * P],
                            wv_sb[:, kc, :],
                            start=(kc == 0), stop=(kc == NT - 1),
                        )
                    nc.vector.tensor_add(vt_sb[:, jm, :], ps[:, :], bv_sb[:, :])

                # next sample's GN stats (DVE; overlaps the ST phase below,
                # ordered after this sample's applies so it can't preempt them)
                mv_next = gn_stats(b + 1, after=last_apply) if b + 1 < NB else None

                # ---- scores transposed + exp ----
                est_sb = estp.tile([P, NJ, HW], bf16, tag="est")
                for jm in range(NJ):
                    st_ps = psb.tile([P, HW], f32, tag="big")
                    for kc in range(NT):
                        lhsT = k_sb[:, kc, jm * P:(jm + 1) * P]
                        st, sp = (kc == 0), (kc == NT - 1)
                        nc.tensor.matmul(st_ps[:, 0:512], lhsT,
                                         q_sb[:, kc, 0:512], start=st, stop=sp)
                        nc.tensor.matmul(st_ps[:, 512:1024], lhsT,
                                         q_sb[:, kc, 512:1024], start=st, stop=sp)
                    nc.scalar.activation(
                        out=est_sb[:, jm, :], in_=st_ps[:, :], func=ACT.Exp,
                        scale=CINV,
                    )

                # ---- softmax denominator ----
                # pairwise f32 tree over the 8 exp tiles (DVE), then one
                # all-ones fp32 matmul reduces the last 128 partitions AND
                # replicates the sum across partitions.
                s0 = sump.tile([P, HW], f32, tag="s0")
                s1 = sump.tile([P, HW], f32, tag="s1")
                s2 = sump.tile([P, HW], f32, tag="s2")
                s3 = sump.tile([P, HW], f32, tag="s3")
                nc.vector.tensor_add(s0[:, :], est_sb[:, 0, :], est_sb[:, 1, :])
                nc.vector.tensor_add(s1[:, :], est_sb[:, 2, :], est_sb[:, 3, :])
                nc.vector.tensor_add(s2[:, :], est_sb[:, 4, :], est_sb[:, 5, :])
                nc.vector.tensor_add(s3[:, :], est_sb[:, 6, :], est_sb[:, 7, :])
                nc.vector.tensor_add(s0[:, :], s0[:, :], s1[:, :])
                nc.vector.tensor_add(s2[:, :], s2[:, :], s3[:, :])
                nc.vector.tensor_add(s0[:, :], s0[:, :], s2[:, :])
                rs0 = psq.tile([P, 512], f32, tag="qkv")
                rs1 = psq.tile([P, 512], f32, tag="qkv")
                nc.tensor.matmul(rs0[:, :], ones_sb[:, :], s0[:, 0:512],
                                 start=True, stop=True)
                nc.tensor.matmul(rs1[:, :], ones_sb[:, :], s0[:, 512:1024],
                                 start=True, stop=True)
                rep = op.tile([P, HW], f32, tag="rep")
                nc.vector.reciprocal_approx_fast(out=rep[:, 0:512], in_=rs0[:, :])
                nc.vector.reciprocal_approx_fast(out=rep[:, 512:1024], in_=rs1[:, :])

                # ---- PV: O[c, i] = (sum_j VT[j, c] * est[j, i]) * r ----
                o_sb = op.tile([P, NT, HW], bf16, tag="o")
                for cm in range(NT):
                    o_ps = psb.tile([P, HW], f32, tag="big")
                    for jm in range(NJ):
                        lhsT = vt_sb[:, jm, cm * P:(cm + 1) * P]
                        st, sp = (jm == 0), (jm == NJ - 1)
                        nc.tensor.matmul(o_ps[:, 0:512], lhsT,
                                         est_sb[:, jm, 0:512], start=st, stop=sp)
                        nc.tensor.matmul(o_ps[:, 512:1024], lhsT,
                                         est_sb[:, jm, 512:1024], start=st, stop=sp)
                    last_oscale = nc.vector.tensor_mul(
                        o_sb[:, cm, :], o_ps[:, :], rep[:, :])

                if mv_next is not None:
                    # ACT is idle during PV/proj: the Sqrt table load no
                    # longer evicts the Exp table mid-score-phase. The next
                    # sample's h applies are also issued here, ahead of this
                    # sample's residuals (but behind the O-scales) on the
                    # DVE stream.
                    gn_affine(b + 1, mv_next)
                    h_cur, last_apply = apply_h(b + 1, after=last_oscale)

                # ---- out-proj + bias + residual ----
                for dm in range(NT):
                    p_ps = psb.tile([P, HW], f32, tag="big")
                    for kc in range(NT):
                        lhsT = wt_sb[:, kc, dm * P:(dm + 1) * P]
                        st, sp = (kc == 0), (kc == NT - 1)
                        nc.tensor.matmul(p_ps[:, 0:512], lhsT,
                                         o_sb[:, kc, 0:512], start=st, stop=sp)
                        nc.tensor.matmul(p_ps[:, 512:1024], lhsT,
                                         o_sb[:, kc, 512:1024], start=st, stop=sp)
                    out_t = outp.tile([P, HW], f32, tag="out")
                    if b == NB - 1:
                        # split the final residuals so the last out-DMAs
                        # start as early as possible
                        for h0, eng in ((0, nc.sync), (512, nc.gpsimd)):
                            nc.vector.scalar_tensor_tensor(
                                out=out_t[:, h0:h0 + 512],
                                in0=p_ps[:, h0:h0 + 512],
                                scalar=bqkt_sb[:, 2, dm:dm + 1],
                                in1=x_sb[:, b, dm, h0:h0 + 512],
                                op0=ALU.add, op1=ALU.add,
                            )
                            eng.dma_start(
                                out=out_d[b, dm * P:(dm + 1) * P, h0:h0 + 512],
                                in_=out_t[:, h0:h0 + 512])
                    else:
                        nc.vector.scalar_tensor_tensor(
                            out=out_t[:, :], in0=p_ps[:, :],
                            scalar=bqkt_sb[:, 2, dm:dm + 1], in1=x_sb[:, b, dm, :],
                            op0=ALU.add, op1=ALU.add,
                        )
                        nc.sync.dma_start(
                            out=out_d[b, dm * P:(dm + 1) * P, 0:512],
                            in_=out_t[:, 0:512])
                        nc.gpsimd.dma_start(
                            out=out_d[b, dm * P:(dm + 1) * P, 512:1024],
                            in_=out_t[:, 512:1024])

    nc.compile()
    return nc


def prep_inputs(inputs):
    """Host-side prep: per-core in_maps with pre-laid-out weights/constants."""
    bf = ml_dtypes.bfloat16
    x = np.ascontiguousarray(np.asarray(inputs["x"], dtype=np.float32)).reshape(
        B_FULL, C, HW
    )

    def wprep(w):
        # [C, C] -> [P, NT, C]  (lhsT slices w[kc*128+p, d])
        return np.ascontiguousarray(
            np.asarray(w, dtype=np.float32).reshape(NT, P, C).transpose(1, 0, 2)
        ).astype(bf)

    def cols(v):
        # [C] -> [P, NT]
        return np.ascontiguousarray(
            np.asarray(v, dtype=np.float32).reshape(NT, P).T
        )

    bqkt = np.stack([cols(inputs["bq"]), cols(inputs["bk"]), cols(inputs["bt"])],
                    axis=1)  # [P, 3, NT]
    gab = np.stack([cols(inputs["gn_scale"]), cols(inputs["gn_bias"])], axis=1)
    bv_rep = np.tile(np.asarray(inputs["bv"], dtype=np.float32)[None, :], (P, 1))
    gg = np.zeros((P, P), np.float32)
    for p in range(P):
        gg[p, (p // GS) * GS:(p // GS + 1) * GS] = 1.0 / GS

    shared = {
        "wq": wprep(inputs["Wq"]), "wk": wprep(inputs["Wk"]),
        "wv": wprep(inputs["Wv"]), "wt": wprep(inputs["Wt"]),
        "bqkt": np.ascontiguousarray(bqkt), "bv_rep": bv_rep,
        "gn_ab": np.ascontiguousarray(gab), "gg": gg,
    }
    in_maps = []
    for c_id in range(N_CORES):
        m = dict(shared)
        m["x"] = np.ascontiguousarray(x[c_id * NB:(c_id + 1) * NB])
        in_maps.append(m)
    return in_maps


_NC_CACHE = {}


def get_nc(zero_qk_bias=True):
    if zero_qk_bias not in _NC_CACHE:
        _NC_CACHE[zero_qk_bias] = build_nc(zero_qk_bias=zero_qk_bias)
    return _NC_CACHE[zero_qk_bias]


def run(inputs, trace=False):
    from concourse.bass_utils import run_bass_kernel_spmd

    zb = bool(
        np.all(np.asarray(inputs["bq"]) == 0) and np.all(np.asarray(inputs["bk"]) == 0)
    )
    nc = get_nc(zero_qk_bias=zb)
    in_maps = prep_inputs(inputs)
    res = run_bass_kernel_spmd(
        nc, in_maps, core_ids=list(range(N_CORES)), trace=trace
    )
    out = np.concatenate([np.asarray(r["out"]) for r in res.results], axis=0)
    return out.reshape(B_FULL, C, H, W), res


def kernel(**inputs):
    out, _ = run(inputs, trace=False)
    return out


# revision 41
# speedup vs baseline: 1.0601x; 1.0601x over previous
"""AttnBlock (GroupNorm -> QKV 1x1 -> full attention over 1024 tokens -> out-proj
+ residual) for x [32, 512, 32, 32] f32, distributed data-parallel over 8
NeuronCores (4 samples per core, weights replicated).

Per-core single-NC Bass/Tile kernel, bf16 TensorE compute, f32 softmax stats:
  - GroupNorm via per-channel bn_stats/bn_aggr + tiny selector matmuls for the
    cross-partition group reduce. Sample b+1's norm chain is issued mid-way
    through sample b's attention so its two tiny PE matmuls embed in the
    matmul stream without stalling it.
  - Q,K: [d, hw] layout; V computed transposed ([hw, d]) directly by swapping
    matmul operands.
  - Scores computed TRANSPOSED (ST[j,i] = sum_d K[d,j] Q[d,i]) so that
    P~ = exp(ST*c) has the contraction axis j on partitions: PV needs no
    transposes. Softmax denominator: DVE pairwise-add tree over the 8 exp
    tiles (f32), then one all-ones fp32 stationary matmul which both reduces
    the remaining 128 partitions and replicates the row-sum across all
    partitions; division is folded into the PV psum->sbuf copy via
    reciprocal_approx_fast.
  - Matmul loops are ordered so consecutive matmuls share the stationary
    operand (one LDWEIGHTS per two matmuls where possible).
"""

import os
import sys

import numpy as np

sys.path.insert(0, "/opt/trn_rl_repo")

import ml_dtypes  # noqa: E402

import concourse.bass as bass  # noqa: E402
import concourse.tile as tile  # noqa: E402
from concourse import bacc, mybir  # noqa: E402

P = 128
B_FULL, C, H, W = 32, 512, 32, 32
HW = H * W            # 1024 tokens
N_CORES = 8
NB = B_FULL // N_CORES  # 4 samples per core
NT = C // P           # 4 channel tiles
NJ = HW // P          # 8 token tiles
NGROUPS = 32
GS = C // NGROUPS     # 16 channels per group
G_PER_TILE = P // GS  # 8 groups per 128-channel tile
EPS = 1e-6
CINV = float(C) ** -0.5

f32 = mybir.dt.float32
bf16 = mybir.dt.bfloat16
ALU = mybir.AluOpType
ACT = mybir.ActivationFunctionType


def build_nc(zero_qk_bias=False):
    """Build the single-core Bass graph (SPMD: same graph on all 8 cores).

    zero_qk_bias: when bq/bk are all-zero (true for this problem's
    setup_inputs), the Q/K psum->sbuf copies use the table-free Copy
    activation instead of Identity+bias, avoiding ACT table thrash with Exp.
    """
    nc = bacc.Bacc("TRN2", target_bir_lowering=False, debug=False)

    x_d = nc.dram_tensor("x", [NB, C, HW], f32, kind="ExternalInput")
    wq_d = nc.dram_tensor("wq", [P, NT, C], bf16, kind="ExternalInput")
    wk_d = nc.dram_tensor("wk", [P, NT, C], bf16, kind="ExternalInput")
    wv_d = nc.dram_tensor("wv", [P, NT, C], bf16, kind="ExternalInput")
    wt_d = nc.dram_tensor("wt", [P, NT, C], bf16, kind="ExternalInput")
    # packed per-partition bias columns: [:, 0, :]=bq, [:, 1, :]=bk, [:, 2, :]=bt
    bqkt_d = nc.dram_tensor("bqkt", [P, 3, NT], f32, kind="ExternalInput")
    bv_d = nc.dram_tensor("bv_rep", [P, C], f32, kind="ExternalInput")
    # gn affine columns: [:, 0, :]=gamma, [:, 1, :]=beta
    gab_d = nc.dram_tensor("gn_ab", [P, 2, NT], f32, kind="ExternalInput")
    # block-diagonal group-average matrix: GG[k,p] = 1/16 iff k//16 == p//16
    gg_d = nc.dram_tensor("gg", [P, P], f32, kind="ExternalInput")
    out_d = nc.dram_tensor("out", [NB, C, HW], f32, kind="ExternalOutput")

    with tile.TileContext(nc) as tc:
        with (
            tc.tile_pool(name="consts", bufs=1) as consts,
            tc.tile_pool(name="hp", bufs=2) as hp,
            tc.tile_pool(name="qkv", bufs=1) as qkvp,
            tc.tile_pool(name="est", bufs=1) as estp,
            tc.tile_pool(name="op", bufs=1) as op,
            tc.tile_pool(name="outp", bufs=3) as outp,
            tc.tile_pool(name="small", bufs=2) as small,
            tc.tile_pool(name="sump", bufs=1) as sump,
            tc.tile_pool(name="psb", bufs=2, space="PSUM") as psb,
            tc.tile_pool(name="psq", bufs=4, space="PSUM") as psq,
        ):
            # ---- x[0] gates everything: one tile per DMA queue (4 parallel
            #      rings), then weights/consts sequenced by first-use time.
            x_sb = consts.tile([P, NB, NT, HW], f32, tag="x")
            wq_sb = consts.tile([P, NT, C], bf16, tag="wq")
            wk_sb = consts.tile([P, NT, C], bf16, tag="wk")
            wv_sb = consts.tile([P, NT, C], bf16, tag="wv")
            wt_sb = consts.tile([P, NT, C], bf16, tag="wt")
            gab_sb = consts.tile([P, 2, NT], f32, tag="gab")
            gg_sb = consts.tile([P, P], f32, tag="gg")
            bqkt_sb = consts.tile([P, 3, NT], f32, tag="bqkt")
            bv_sb = consts.tile([P, C], f32, tag="bv")

            engs = (nc.sync, nc.gpsimd, nc.scalar)
            qi = 0
            for t in range(NT):
                for h0 in (0, 256, 512, 768):
                    engs[qi % 3].dma_start(
                        out=x_sb[:, 0, t, h0:h0 + 256],
                        in_=x_d[0, t * P:(t + 1) * P, h0:h0 + 256])
                    qi += 1
            # sync queue: wq, then x[1], x[3]
            nc.sync.dma_start(out=wq_sb[:, :, :], in_=wq_d[:, :, :])
            for b in (1, 3):
                for t in range(NT):
                    nc.sync.dma_start(out=x_sb[:, b, t, :],
                                      in_=x_d[b, t * P:(t + 1) * P, :])
            # gpsimd queue: GN consts (needed ~15us), wk (~20us), x[2],
            # then wv (~35us), bv, wt (~45us)
            nc.gpsimd.dma_start(out=gg_sb[:, :], in_=gg_d[:, :])
            nc.gpsimd.dma_start(out=gab_sb[:, :, :], in_=gab_d[:, :, :])
            nc.gpsimd.dma_start(out=bqkt_sb[:, :, :], in_=bqkt_d[:, :, :])
            nc.gpsimd.dma_start(out=wk_sb[:, :, :], in_=wk_d[:, :, :])
            for t in range(NT):
                nc.gpsimd.dma_start(out=x_sb[:, 2, t, :],
                                    in_=x_d[2, t * P:(t + 1) * P, :])
            nc.gpsimd.dma_start(out=wv_sb[:, :, :], in_=wv_d[:, :, :])
            nc.gpsimd.dma_start(out=bv_sb[:, :], in_=bv_d[:, :])
            nc.gpsimd.dma_start(out=wt_sb[:, :, :], in_=wt_d[:, :, :])
            ones_sb = consts.tile([P, P], f32, tag="ones")
            nc.vector.memset(ones_sb[:, :], 1.0)
            magic_sb = consts.tile([P, NT], mybir.dt.int32, tag="magic")
            nc.vector.memset(magic_sb[:, :], 0x5F3759DF)

            a_all = consts.tile([P, NB, NT], f32, tag="a_all")
            b_all = consts.tile([P, NB, NT], f32, tag="b_all")

            # PE warm-up: harmless fp32 matmuls on the earliest-arriving x
            # tile so the HAM clock-gate is released before the real stream.
            warm_ps = psq.tile([P, 512], f32, tag="qkv")
            for w in range(5):
                nc.tensor.matmul(
                    warm_ps[:, :], x_sb[:, 0, 0, 0:128], x_sb[:, 0, 0, 0:512],
                    start=(w == 0), stop=(w == 4),
                )

            def gn_stats(b, after=None):
                """bn stats -> per-channel (mean, Ex2) packed in mv."""
                mv = small.tile([P, NT, 2], f32, tag="mv")
                for t in range(NT):
                    st6 = small.tile([P, 2, 6], f32, tag="st6")
                    i0 = nc.vector.bn_stats(out=st6[:, 0, :], in_=x_sb[:, b, t, 0:512])
                    i1 = nc.vector.bn_stats(out=st6[:, 1, :], in_=x_sb[:, b, t, 512:1024])
                    nc.vector.bn_aggr(out=mv[:, t, :], in_=st6[:, :, :])
                    if after is not None:
                        # keep these off the DVE stream until the previous
                        # sample's applies have been scheduled
                        tile.add_dep_helper(i0.ins, after.ins, sync=False,
                                            reason="gn stats after prev apply")
                        tile.add_dep_helper(i1.ins, after.ins, sync=False,
                                            reason="gn stats after prev apply")
                msq = small.tile([P, NT], f32, tag="msq")
                nc.vector.tensor_mul(msq[:, :], mv[:, :, 0], mv[:, :, 0])
                nc.vector.tensor_add(mv[:, :, 1], mv[:, :, 1], msq[:, :])
                return mv

            def gn_affine(b, mv, use_act_sqrt=False):
                """fused group-avg+broadcast matmul, then form per-channel A/B."""
                bc_ps = psq.tile([P, 512], f32, tag="qkv")
                nc.tensor.matmul(bc_ps[:, :NT * 2], gg_sb[:, :], mv[:, :, :],
                                 start=True, stop=True)
                bc = small.tile([P, NT, 2], f32, tag="bcs")
                nc.vector.tensor_copy(bc[:, :, :], bc_ps[:, 0:NT * 2])
                vb = small.tile([P, NT], f32, tag="vb")
                nc.vector.tensor_mul(vb[:, :], bc[:, :, 0], bc[:, :, 0])
                nc.vector.tensor_sub(vb[:, :], bc[:, :, 1], vb[:, :])
                if use_act_sqrt:
                    # sample 0 (pre-exp): the shorter ACT chain wins and its
                    # Sqrt table load cannot evict a not-yet-loaded Exp table
                    nc.vector.tensor_scalar_add(vb[:, :], vb[:, :], EPS)
                    nc.scalar.sqrt(vb[:, :], vb[:, :])
                    nc.vector.reciprocal(vb[:, :], vb[:, :])
                    y1 = vb
                else:
                    nc.vector.tensor_scalar_add(vb[:, :], vb[:, :], EPS)
                    # rstd = rsqrt(var+eps): fast-inverse-sqrt + 2 Newton steps
                    # (all-DVE: keeps Sqrt off ACT so it never evicts Exp)
                    ii = small.tile([P, NT], mybir.dt.int32, tag="ii")
                    nc.vector.tensor_scalar(
                        out=ii[:, :], in0=vb.bitcast(mybir.dt.int32)[:, :],
                        scalar1=1, scalar2=None, op0=ALU.arith_shift_right)
                    nc.vector.tensor_tensor(ii[:, :], magic_sb[:, :], ii[:, :],
                                            op=ALU.subtract)
                    y0 = ii.bitcast(f32)
                    yt = small.tile([P, NT], f32, tag="yt")
                    y1 = small.tile([P, NT], f32, tag="y1")
                    nc.vector.tensor_mul(yt[:, :], vb[:, :], y0[:, :])
                    nc.vector.tensor_mul(yt[:, :], yt[:, :], y0[:, :])
                    nc.vector.tensor_scalar(out=yt[:, :], in0=yt[:, :], scalar1=-0.5,
                                            scalar2=1.5, op0=ALU.mult, op1=ALU.add)
                    nc.vector.tensor_mul(y1[:, :], y0[:, :], yt[:, :])
                    nc.vector.tensor_mul(yt[:, :], vb[:, :], y1[:, :])
                    nc.vector.tensor_mul(yt[:, :], yt[:, :], y1[:, :])
                    nc.vector.tensor_scalar(out=yt[:, :], in0=yt[:, :], scalar1=-0.5,
                                            scalar2=1.5, op0=ALU.mult, op1=ALU.add)
                    nc.vector.tensor_mul(y1[:, :], y1[:, :], yt[:, :])
                tmp = small.tile([P, NT], f32, tag="tmpab")
                nc.vector.tensor_mul(a_all[:, b, :], y1[:, :], gab_sb[:, 0, :])
                nc.vector.tensor_mul(tmp[:, :], bc[:, :, 0], a_all[:, b, :])
                nc.vector.tensor_sub(b_all[:, b, :], gab_sb[:, 1, :], tmp[:, :])

            def apply_h(b, after=None):
                """h = x*A + B (bf16)"""
                h = hp.tile([P, NT, HW], bf16, tag="h")
                last = None
                for t in range(NT):
                    last = nc.vector.tensor_scalar(
                        out=h[:, t, :], in0=x_sb[:, b, t, :],
                        scalar1=a_all[:, b, t:t + 1], scalar2=b_all[:, b, t:t + 1],
                        op0=ALU.mult, op1=ALU.add,
                    )
                    if after is not None:
                        # the applies must not jump ahead of this sample's
                        # O-scales on the in-order DVE stream
                        tile.add_dep_helper(last.ins, after.ins, sync=False,
                                            reason="apply after O-scales")
                return h, last

            mv0 = gn_stats(0)
            gn_affine(0, mv0, use_act_sqrt=True)
            h_cur, last_apply = apply_h(0)

            for b in range(NB):
                h_sb = h_cur

                # ---- Q, K projections: [d, hw]; lhsT reused across n-halves ----
                q_sb = qkvp.tile([P, NT, HW], bf16, tag="q")
                k_sb = qkvp.tile([P, NT, HW], bf16, tag="k")
                # Q psum->sbuf copies on ACT (overlap the K matmuls); K copies
                # on the then-idle DVE so ACT's exp stream starts immediately
                # once the score matmuls begin.
                for dst_sb, w_sb, bias_idx in ((q_sb, wq_sb, 0), (k_sb, wk_sb, 1)):
                    for dm in range(NT):
                        ps0 = psq.tile([P, 512], f32, tag="qkv")
                        ps1 = psq.tile([P, 512], f32, tag="qkv")
                        for kc in range(NT):
                            lhsT = w_sb[:, kc, dm * P:(dm + 1) * P]
                            st, sp = (kc == 0), (kc == NT - 1)
                            nc.tensor.matmul(ps0[:, :], lhsT,
                                             h_sb[:, kc, 0:512], start=st, stop=sp)
                            nc.tensor.matmul(ps1[:, :], lhsT,
                                             h_sb[:, kc, 512:1024], start=st, stop=sp)
                        for ps, nsl in ((ps0, slice(0, 512)), (ps1, slice(512, 1024))):
                            if zero_qk_bias:
                                nc.scalar.copy(dst_sb[:, dm, nsl], ps[:, :])
                            else:
                                nc.scalar.add(dst_sb[:, dm, nsl], ps[:, :],
                                              bqkt_sb[:, bias_idx, dm:dm + 1])
                # ---- V transposed: VT[j, d] ----
                vt_sb = qkvp.tile([P, NJ, C], bf16, tag="vt")
                for jm in range(NJ):
                    ps = psq.tile([P, 512], f32, tag="qkv")
                    for kc in range(NT):
                        nc.tensor.matmul(
                            ps[:, :],
                            h_sb[:, kc, jm * P:(jm + 1) * P],
                            wv_sb[:, kc, :],
                            start=(kc == 0), stop=(kc == NT - 1),
                        )
                    nc.vector.tensor_add(vt_sb[:, jm, :], ps[:, :], bv_sb[:, :])

                # next sample's GN stats (DVE; overlaps the ST phase below,
                # ordered after this sample's applies so it can't preempt them)
                mv_next = gn_stats(b + 1, after=last_apply) if b + 1 < NB else None

                # ---- scores transposed + exp ----
                est_sb = estp.tile([P, NJ, HW], bf16, tag="est")
                for jm in range(NJ):
                    st_ps = psb.tile([P, HW], f32, tag="big")
                    for kc in range(NT):
                        lhsT = k_sb[:, kc, jm * P:(jm + 1) * P]
                        st, sp = (kc == 0), (kc == NT - 1)
                        nc.tensor.matmul(st_ps[:, 0:512], lhsT,
                                         q_sb[:, kc, 0:512], start=st, stop=sp)
                        nc.tensor.matmul(st_ps[:, 512:1024], lhsT,
                                         q_sb[:, kc, 512:1024], start=st, stop=sp)
                    nc.scalar.activation(
                        out=est_sb[:, jm, :], in_=st_ps[:, :], func=ACT.Exp,
                        scale=CINV,
                    )

                # ---- softmax denominator ----
                # pairwise f32 tree over the 8 exp tiles (DVE), then one
                # all-ones fp32 matmul reduces the last 128 partitions AND
                # replicates the sum across partitions.
                s0 = sump.tile([P, HW], f32, tag="s0")
                s1 = sump.tile([P, HW], f32, tag="s1")
                s2 = sump.tile([P, HW], f32, tag="s2")
                s3 = sump.tile([P, HW], f32, tag="s3")
                nc.vector.tensor_add(s0[:, :], est_sb[:, 0, :], est_sb[:, 1, :])
                nc.vector.tensor_add(s1[:, :], est_sb[:, 2, :], est_sb[:, 3, :])
                nc.vector.tensor_add(s2[:, :], est_sb[:, 4, :], est_sb[:, 5, :])
                nc.vector.tensor_add(s3[:, :], est_sb[:, 6, :], est_sb[:, 7, :])
                nc.vector.tensor_add(s0[:, :], s0[:, :], s1[:, :])
                nc.vector.tensor_add(s2[:, :], s2[:, :], s3[:, :])
                nc.vector.tensor_add(s0[:, :], s0[:, :], s2[:, :])
                rs0 = psq.tile([P, 512], f32, tag="qkv")
                rs1 = psq.tile([P, 512], f32, tag="qkv")
                nc.tensor.matmul(rs0[:, :], ones_sb[:, :], s0[:, 0:512],
                                 start=True, stop=True)
                nc.tensor.matmul(rs1[:, :], ones_sb[:, :], s0[:, 512:1024],
                                 start=True, stop=True)
                rep = op.tile([P, HW], f32, tag="rep")
                nc.vector.reciprocal_approx_fast(out=rep[:, 0:512], in_=rs0[:, :])
                nc.vector.reciprocal_approx_fast(out=rep[:, 512:1024], in_=rs1[:, :])

                # ---- PV: O[c, i] = (sum_j VT[j, c] * est[j, i]) * r ----
                o_sb = op.tile([P, NT, HW], bf16, tag="o")
                for cm in range(NT):
                    o_ps = psb.tile([P, HW], f32, tag="big")
                    for jm in range(NJ):
                        lhsT = vt_sb[:, jm, cm * P:(cm + 1) * P]
                        st, sp = (jm == 0), (jm == NJ - 1)
                        nc.tensor.matmul(o_ps[:, 0:512], lhsT,
                                         est_sb[:, jm, 0:512], start=st, stop=sp)
                        nc.tensor.matmul(o_ps[:, 512:1024], lhsT,
                                         est_sb[:, jm, 512:1024], start=st, stop=sp)
                    last_oscale = nc.vector.tensor_mul(
                        o_sb[:, cm, :], o_ps[:, :], rep[:, :])

                if mv_next is not None:
                    # ACT is idle during PV/proj: the Sqrt table load no
                    # longer evicts the Exp table mid-score-phase. The next
                    # sample's h applies are also issued here, ahead of this
                    # sample's residuals (but behind the O-scales) on the
                    # DVE stream.
                    gn_affine(b + 1, mv_next)
                    h_cur, last_apply = apply_h(b + 1, after=last_oscale)

                # ---- out-proj + bias + residual ----
                for dm in range(NT):
                    p_ps = psb.tile([P, HW], f32, tag="big")
                    for kc in range(NT):
                        lhsT = wt_sb[:, kc, dm * P:(dm + 1) * P]
                        st, sp = (kc == 0), (kc == NT - 1)
                        nc.tensor.matmul(p_ps[:, 0:512], lhsT,
                                         o_sb[:, kc, 0:512], start=st, stop=sp)
                        nc.tensor.matmul(p_ps[:, 512:1024], lhsT,
                                         o_sb[:, kc, 512:1024], start=st, stop=sp)
                    out_t = outp.tile([P, HW], f32, tag="out")
                    if b == NB - 1:
                        # split the final residuals so the last out-DMAs
                        # start as early as possible
                        for h0, eng in ((0, nc.sync), (512, nc.gpsimd)):
                            nc.vector.scalar_tensor_tensor(
                                out=out_t[:, h0:h0 + 512],
                                in0=p_ps[:, h0:h0 + 512],
                                scalar=bqkt_sb[:, 2, dm:dm + 1],
                                in1=x_sb[:, b, dm, h0:h0 + 512],
                                op0=ALU.add, op1=ALU.add,
                            )
                            eng.dma_start(
                                out=out_d[b, dm * P:(dm + 1) * P, h0:h0 + 512],
                                in_=out_t[:, h0:h0 + 512])
                    else:
                        nc.vector.scalar_tensor_tensor(
                            out=out_t[:, :], in0=p_ps[:, :],
                            scalar=bqkt_sb[:, 2, dm:dm + 1], in1=x_sb[:, b, dm, :],
                            op0=ALU.add, op1=ALU.add,
                        )
                        nc.sync.dma_start(
                            out=out_d[b, dm * P:(dm + 1) * P, 0:512],
                            in_=out_t[:, 0:512])
                        nc.gpsimd.dma_start(
                            out=out_d[b, dm * P:(dm + 1) * P, 512:1024],
                            in_=out_t[:, 512:1024])

    nc.compile()
    return nc


def prep_inputs(inputs):
    """Host-side prep: per-core in_maps with pre-laid-out weights/constants."""
    bf = ml_dtypes.bfloat16
    x = np.ascontiguousarray(np.asarray(inputs["x"], dtype=np.float32)).reshape(
        B_FULL, C, HW
    )

    def wprep(w):
        # [C, C] -> [P, NT, C]  (lhsT slices w[kc*128+p, d])
        return np.ascontiguousarray(
            np.asarray(w, dtype=np.float32).reshape(NT, P, C).transpose(1, 0, 2)
        ).astype(bf)

    def cols(v):
        # [C] -> [P, NT]
        return np.ascontiguousarray(
            np.asarray(v, dtype=np.float32).reshape(NT, P).T
        )

    bqkt = np.stack([cols(inputs["bq"]), cols(inputs["bk"]), cols(inputs["bt"])],
                    axis=1)  # [P, 3, NT]
    gab = np.stack([cols(inputs["gn_scale"]), cols(inputs["gn_bias"])], axis=1)
    bv_rep = np.tile(np.asarray(inputs["bv"], dtype=np.float32)[None, :], (P, 1))
    gg = np.zeros((P, P), np.float32)
    for p in range(P):
        gg[p, (p // GS) * GS:(p // GS + 1) * GS] = 1.0 / GS

    shared = {
        "wq": wprep(inputs["Wq"]), "wk": wprep(inputs["Wk"]),
        "wv": wprep(inputs["Wv"]), "wt": wprep(inputs["Wt"]),
        "bqkt": np.ascontiguousarray(bqkt), "bv_rep": bv_rep,
        "gn_ab": np.ascontiguousarray(gab), "gg": gg,
    }
    in_maps = []
    for c_id in range(N_CORES):
        m = dict(shared)
        m["x"] = np.ascontiguousarray(x[c_id * NB:(c_id + 1) * NB])
        in_maps.append(m)
    return in_maps


_NC_CACHE = {}


def get_nc(zero_qk_bias=True):
    if zero_qk_bias not in _NC_CACHE:
        _NC_CACHE[zero_qk_bias] = build_nc(zero_qk_bias=zero_qk_bias)
    return _NC_CACHE[zero_qk_bias]


def run(inputs, trace=False):
    from concourse.bass_utils import run_bass_kernel_spmd

    zb = bool(
        np.all(np.asarray(inputs["bq"]) == 0) and np.all(np.asarray(inputs["bk"]) == 0)
    )
    nc = get_nc(zero_qk_bias=zb)
    in_maps = prep_inputs(inputs)
    res = run_bass_kernel_spmd(
        nc, in_maps, core_ids=list(range(N_CORES)), trace=trace
    )
    out = np.concatenate([np.asarray(r["out"]) for r in res.results], axis=0)
    return out.reshape(B_FULL, C, H, W), res


def kernel(**inputs):
    out, _ = run(inputs, trace=False)
    return out
